# revision 1
# baseline (speedup 1.0000x reference)
"""Trainium2 Bass kernel for nn_Eq1dConv (conv1d(K=3)+bias -> filtered_lrelu).

Math (separable along W; H is untouched because the 2x up/down in H uses a
1-tap filter, so inserted zero rows are dropped again by the ::2 decimate):

  y_b[co,h,m]  = sum_{ci,k} x[ci,h,m+k-1]*w[co,ci,k] + b[co]      (m in [0,512))
  pre_a[m'] = fk1*(y_b[m'-1]+y_b[m'])                  (up-FIR even phase, fk1==fk3)
  pre_b[m'] = fk0*(y_b[m'-1]+y_b[m'+1]) + fk2*y_b[m']  (odd phase, fk0==fk4)
  out[n] = fd0*lr(pre_a[n]) + fd1*lr(pre_b[n]) + fd2*lr(pre_a[n+1]) + fd3*lr(pre_b[n+1])

with lr = leaky-relu(0.2), fk = 4*flip(up_filter), fd = flip(down_filter).
lr(c*u) = c*max(u,0.2u) for c>0 (min for c<0), so the fk/fd scales fold into
the diagonal matmuls of the final comb.

Implementation notes (all driven by HW probes):
- fp16 everywhere on-chip (matmuls 1 cyc/row; DVE 16-bit 2x needs 4B-aligned
  even-element offsets; fp16 keeps 2^-11 precision).
- Per row-pair (h, h+32), partitions q=2*ci+g / 2*co+g: 3 matmuls -> y PSUM.
- ONE ACT op evicts y into TWO shifted fp16 rows (A: y_b[j-1], B: y_b[j]) of a
  persistent padded buffer, adding the per-partition bias (bias only lands on
  valid columns => boundary zeros handled for free).
- DVE/GPSIMD stages batched over a whole granule (4 row-pairs) to amortize
  fixed costs; all 16-bit shifts are even (aligned) by construction.
- scalar_tensor_tensor is 1x-only; tensor_tensor 16-bit aligned is 2x;
  tensor_scalar is 4x; gpsimd tensor ops ~2.1 cyc/elem (SBUF only).

Sharding: pure data-parallel, batch 8 -> 8 cores, weights replicated.
"""

import numpy as np
from contextlib import ExitStack

import concourse.bass as bass
import concourse.bacc as bacc
import concourse.mybir as mybir
import concourse.tile as tile
from concourse.bass_utils import run_bass_kernel_spmd

B, CIN, COUT, H, W, K = 8, 64, 64, 64, 512, 3
N_CORES = 8
SLOPE = 0.2

F32 = mybir.dt.float32
F16 = mybir.dt.float16
ADD = mybir.AluOpType.add
MULT = mybir.AluOpType.mult


def _alu(c):
    # lr(c*u) = c * (max if c > 0 else min)(u, SLOPE*u)
    return mybir.AluOpType.max if c > 0 else mybir.AluOpType.min


def build_program(n_rowpairs=H // 2, rp_per_gran=4):
    """Build the single-core SPMD program. Returns (nc, go)."""
    nc = bacc.Bacc("TRN2", target_bir_lowering=False, debug=False)

    x_d = nc.declare_dram_parameter("x", [CIN, H, W], F32, isOutput=False)
    wb_d = nc.declare_dram_parameter("wb", [K, 128, 128], F16, isOutput=False)
    bcol_d = nc.declare_dram_parameter("bcol", [128, 1], F32, isOutput=False)
    dg_d = nc.declare_dram_parameter("dg", [4, 128, 128], F16, isOutput=False)
    out_d = nc.declare_dram_parameter("out", [COUT, H, W], F32, isOutput=True)

    n_gran = (n_rowpairs + rp_per_gran - 1) // rp_per_gran
    NYB = 2  # yy double-buffer count
    YW = 520  # per-row width of the shifted-y buffer (pads included)

    def go(ratio, alu_a, alu_b):
        with tile.TileContext(nc) as tc, ExitStack() as ctx:
            cpool = ctx.enter_context(tc.tile_pool(name="consts", bufs=1))
            xpool = ctx.enter_context(tc.tile_pool(name="xg", bufs=2))
            opool = ctx.enter_context(tc.tile_pool(name="og", bufs=2))
            ypool = ctx.enter_context(
                tc.tile_pool(name="ypsum", bufs=1, space=bass.MemorySpace.PSUM)
            )
            fpool = ctx.enter_context(
                tc.tile_pool(name="fpsum", bufs=1, space=bass.MemorySpace.PSUM)
            )
            wkpool = ctx.enter_context(tc.tile_pool(name="work", bufs=2))

            wb_t = []
            for k in range(K):
                t = cpool.tile([128, 128], F16, tag=f"wb{k}")
                nc.sync.dma_start(t[:], wb_d[k])
                wb_t.append(t)
            dg_t = []
            for k in range(4):
                t = cpool.tile([128, 128], F16, tag=f"dg{k}")
                nc.sync.dma_start(t[:], dg_d[k])
                dg_t.append(t)
            bcol = cpool.tile([128, 1], F32, tag="bcol")
            nc.sync.dma_start(bcol[:], bcol_d[:])

            # persistent shifted-y buffers: [128, rp, 2, YW]
            #   row 0 (A): col j = y_b[j-1]  (valid j in [1,513), pads zero)
            #   row 1 (B): col j = y_b[j]    (valid j in [0,512), pads zero)
            yybufs = []
            for i in range(NYB):
                t = cpool.tile([128, rp_per_gran, 2, YW], F16, tag=f"yy{i}")
                nc.vector.memset(t[:, :, 0, 0:1], 0.0)
                nc.vector.memset(t[:, :, 0, 513:YW], 0.0)
                nc.vector.memset(t[:, :, 1, 512:YW], 0.0)
                yybufs.append(t)

            mm = lambda o_, l_, r_, s1, s2: nc.tensor.matmul(
                o_, l_, r_, start=s1, stop=s2
            )

            x_view = x_d.rearrange("c (p hh) w -> (c p) hh w", p=2)
            o_view = out_d.rearrange("c (p hh) w -> (c p) hh w", p=2)

            for g in range(n_gran):
                rp0 = g * rp_per_gran
                nrp = min(rp_per_gran, n_rowpairs - rp0)
                xg = xpool.tile([128, rp_per_gran, W], F16, tag="xg")
                # SWDGE dma casts f32 -> f16 in flight
                nc.gpsimd.dma_start(xg[:, 0:nrp, :], x_view[:, rp0 : rp0 + nrp, :])
                og = opool.tile([128, rp_per_gran, W], F32, tag="og")
                yy = yybufs[g % NYB]

                # tap-outer ordering: each weight loaded once per granule,
                # row-pair matmuls run back-to-back into 4 separate y banks
                y_t = [ypool.tile([128, 512], F32, tag=f"y{j}", name=f"y{j}") for j in range(nrp)]
                for j in range(nrp):  # k=1 (widest range, starts the groups)
                    mm(y_t[j][:, 0:512], wb_t[1][:], xg[:, j, 0:512], True, False)
                for j in range(nrp):  # k=0
                    mm(y_t[j][:, 1:512], wb_t[0][:], xg[:, j, 0:511], False, False)
                for j in range(nrp):  # k=2 (stops the groups)
                    mm(y_t[j][:, 0:511], wb_t[2][:], xg[:, j, 1:512], False, True)
                for j in range(nrp):
                    # ONE ACT op: y+bias -> both shifted fp16 rows of yy[:, j]
                    flat = yy[:, j, :, :].rearrange("p a b -> p (a b)")
                    dual = flat[:, 1 : 1 + 2 * 519].rearrange(
                        "p (r c) -> p r c", c=519
                    )[:, :, 0:512]
                    src = y_t[j][:, 0:512].unsqueeze(1).broadcast_to([128, 2, 512])
                    nc.scalar.activation(
                        dual,
                        src,
                        mybir.ActivationFunctionType.Identity,
                        bias=bcol[:, 0:1],
                        scale=1.0,
                    )

                # batched vector stages over the granule
                nj = nrp
                yA = yy[:, 0:nj, 0, :]
                yB = yy[:, 0:nj, 1, :]
                s_a = wkpool.tile([128, rp_per_gran, 513], F16, tag="s_a")
                nc.vector.tensor_tensor(
                    s_a[:, 0:nj, :], yA[:, :, 0:513], yB[:, :, 0:513], ADD
                )
                s_b0 = wkpool.tile([128, rp_per_gran, 513], F16, tag="s_b0")
                nc.gpsimd.tensor_tensor(
                    s_b0[:, 0:nj, :], yA[:, :, 0:513], yA[:, :, 2:515], ADD
                )
                # u = ratio*yB + s_b0: 4x TS then 2x TT-add (faster than 1x STT)
                t = wkpool.tile([128, rp_per_gran, 513], F16, tag="t")
                nc.vector.tensor_scalar(
                    t[:, 0:nj, :], yB[:, :, 0:513], float(ratio), None, MULT
                )
                u = wkpool.tile([128, rp_per_gran, 513], F16, tag="u")
                nc.vector.tensor_tensor(
                    u[:, 0:nj, :], t[:, 0:nj, :], s_b0[:, 0:nj, :], ADD
                )
                # lrelu cores via fused STT (1x but single-op; TT-max is slower)
                b2 = wkpool.tile([128, rp_per_gran, 513], F16, tag="b2")
                nc.vector.scalar_tensor_tensor(
                    b2[:, 0:nj, :], u[:, 0:nj, :], SLOPE, u[:, 0:nj, :], MULT, alu_b
                )
                a2 = wkpool.tile([128, rp_per_gran, 513], F16, tag="a2")
                nc.vector.scalar_tensor_tensor(
                    a2[:, 0:nj, :], s_a[:, 0:nj, :], SLOPE, s_a[:, 0:nj, :], MULT, alu_a
                )

                f_t = [fpool.tile([128, 512], F32, tag=f"f{j}", name=f"f{j}") for j in range(nrp)]
                for j in range(nrp):
                    mm(f_t[j][:], dg_t[0][:], a2[:, j, 0:512], True, False)
                for j in range(nrp):
                    mm(f_t[j][:], dg_t[1][:], b2[:, j, 0:512], False, False)
                for j in range(nrp):
                    mm(f_t[j][:], dg_t[2][:], a2[:, j, 1:513], False, False)
                for j in range(nrp):
                    mm(f_t[j][:], dg_t[3][:], b2[:, j, 1:513], False, True)
                for j in range(nrp):
                    nc.scalar.copy(og[:, j, :], f_t[j][:])

                nc.sync.dma_start(o_view[:, rp0 : rp0 + nrp, :], og[:, 0:nrp, :])

    return nc, go


def derive_consts(conv_w, bias, up_filter, down_filter):
    f = np.asarray(up_filter, dtype=np.float64).reshape(-1)
    d = np.asarray(down_filter, dtype=np.float64).reshape(-1)
    fk = (f * 4.0)[::-1]
    fd = d[::-1]
    assert abs(fk[1] - fk[3]) < 1e-6 * max(1.0, abs(fk[1])), "up filter not symmetric"
    assert abs(fk[0] - fk[4]) < 1e-6 * max(1.0, abs(fk[0])), "up filter not symmetric"
    fk0, fk1, fk2 = float(fk[0]), float(fk[1]), float(fk[2])
    assert fk0 != 0.0
    ratio = fk2 / fk0

    # partition index q = 2*ci + g (g = h-half); output partition 2*co + g
    cw = np.asarray(conv_w, dtype=np.float32)  # [co, ci, 1, K]
    wb = np.zeros((K, 128, 128), dtype=np.float16)
    for k in range(K):
        wk = cw[:, :, 0, k].T.astype(np.float16)  # [ci, co]
        wb[k, 0::2, 0::2] = wk
        wb[k, 1::2, 1::2] = wk

    bcol = np.repeat(np.asarray(bias, dtype=np.float32), 2).reshape(128, 1)

    eye = np.eye(128, dtype=np.float32)
    dg = np.stack(
        [
            np.float32(fd[0] * fk1) * eye,
            np.float32(fd[1] * fk0) * eye,
            np.float32(fd[2] * fk1) * eye,
            np.float32(fd[3] * fk0) * eye,
        ]
    ).astype(np.float16)

    return {
        "wb": wb,
        "bcol": bcol,
        "dg": dg,
        "ratio": ratio,
        "alu_a": _alu(fk1),
        "alu_b": _alu(fk0),
    }


_CACHE = {}


def _get_compiled(consts_key, ratio, alu_a, alu_b):
    if consts_key in _CACHE:
        return _CACHE[consts_key]
    nc, go = build_program()
    go(ratio, alu_a, alu_b)
    nc.compile()
    _CACHE[consts_key] = nc
    return nc


def run(x, conv_w, bias, up_filter, down_filter, trace=False, **trace_kw):
    x = np.asarray(x, dtype=np.float32)
    c = derive_consts(conv_w, bias, up_filter, down_filter)

    key = (float(c["ratio"]), c["alu_a"].value, c["alu_b"].value)
    nc = _get_compiled(key, c["ratio"], c["alu_a"], c["alu_b"])

    in_maps = []
    for i in range(N_CORES):
        in_maps.append(
            {
                "x": np.ascontiguousarray(x[i]),
                "wb": c["wb"],
                "bcol": c["bcol"],
                "dg": c["dg"],
            }
        )
    res = run_bass_kernel_spmd(
        nc, in_maps, list(range(N_CORES)), trace=trace, **trace_kw
    )
    out = np.stack([res.results[i]["out"] for i in range(N_CORES)], axis=0)
    return out.astype(np.float32), res


def kernel(x, conv_w, bias, up_filter, down_filter):
    out, _ = run(x, conv_w, bias, up_filter, down_filter)
    return out

